# revision 36
# baseline (speedup 1.0000x reference)
"""Barnes-Wall (BW16) lattice quantizer for Trainium2, 8-core data-parallel.

Fast-Hadamard reformulation (validated bit-exact vs the jax reference in numpy):
  Per coordinate there are only two rounded candidates: the nearest even-lattice
  point E = 2*RNE(x/2) (error eE = E - x) and the nearest odd point O = E + dEO.
  Codeword k selects E or O per coordinate via its parity pattern b_k, and the 32
  patterns form the RM(1,4) code: with columns permuted so the code labels are
  position bits, b_k[p] = s XOR <m, p>.  Dropping row-common terms, the squared
  distance is sgn_s * WHT_m(|eE|-1/2), the rounded vector's parity comes from
  WHT_m(dEO), and the parity-repair penalty max|e| from max/min half-space
  tables built with a max-butterfly.  No [*,32,16] tensors anywhere.

Layout: butterfly/assembly tensors are stored transposed as [lane, R] with the
R row-blocks contiguous innermost, so every engine instruction streams runs of
R elements regardless of lane-shuffle strides.
"""
import os
import sys

sys.path.insert(0, "/opt/trn_rl_repo")
import contextlib

KSTAGE = int(os.environ.get("KSTAGE", "99"))  # debug bisection stage

import numpy as np

import concourse.bass as bass
import concourse.bacc as bacc
import concourse.mybir as mybir
import concourse.tile as tile

f32 = np.float32
MAGIC1 = float(f32(1.5 * 2.0**23))  # round-to-nearest-integer magic

dt = mybir.dt
Alu = mybir.AluOpType
Act = mybir.ActivationFunctionType
AX = mybir.AxisListType

N_CORES = 8
R = 32  # row blocks per iteration (rows/iter = 128*R)
BIGNEG = -1.0e30

# ---- host constants (derived from the fixed BW16 codebook; see module docstring)
_G = np.array([
    [1,1,1,1,0,1,0,1,1,0,0,1,0,0,0,0],
    [0,1,1,1,1,0,1,0,1,1,0,0,1,0,0,0],
    [0,0,1,1,1,1,0,1,0,1,1,0,0,1,0,0],
    [0,0,0,1,1,1,1,0,1,0,1,1,0,0,1,0],
    [1,1,1,1,1,1,1,1,1,1,1,1,1,1,1,1]], dtype=np.int64)


def _host_consts():
    import itertools
    G = _G
    bits_all = np.array(list(itertools.product([0, 1], repeat=5)), dtype=np.int64)
    Ci = bits_all @ G
    Bp = Ci % 2
    v = (G[0] + G[1] * 2 + G[2] * 4 + G[3] * 8)
    jinv = np.zeros(16, dtype=np.int64)
    for j in range(16):
        jinv[v[j]] = j
    bitrev = np.array([int(f"{m:04b}"[::-1], 2) for m in range(16)])
    orig_r = np.zeros(32, dtype=np.int64)
    for k in range(32):
        s, mt = divmod(k, 16)
        m = bitrev[mt]
        orig_r[k] = 16 * (m & 1) + 8 * ((m >> 1) & 1) + 4 * ((m >> 2) & 1) + 2 * ((m >> 3) & 1) + s
    Tk = np.array([int(np.sum(np.where(Bp[r] == 1, (Ci[r] - 1) // 2, Ci[r] // 2))) for r in range(32)])
    sgn32 = (1.0 - 2.0 * (np.arange(32) // 16)).astype(np.float32)
    pcq32 = (sgn32 * 0.25).astype(np.float32)
    ckq32 = np.array([
        Tk[orig_r[k]] + 4 - 4 * (1 - 2 * (k // 16)) * (1 if k % 16 == 0 else 0) + 256
        for k in range(32)], dtype=np.float32)
    # CC packs (original rank, permuted-codeword bits) so one masked-min both
    # breaks ties by original rank and yields the winner's bit pattern.
    bw = np.zeros(32, dtype=np.int64)
    for k in range(32):
        s, mt = divmod(k, 16)
        m = bitrev[mt]
        for p in range(16):
            if (s + bin(m & p).count("1")) % 2 == 1:
                bw[k] += 1 << p
    ccc = (orig_r * 65536 + bw - 2200000).astype(np.float32)
    i16c = (jinv - 32).astype(np.float32)
    pshift = np.arange(16, dtype=np.int32)
    return jinv, sgn32, pcq32, ckq32, ccc, i16c, pshift


JINV, SGN32, PCQ32, CKQ32, CCC, I16C, PSHIFT = _host_consts()


def _div_consts(a_val):
    r64 = 1.0 / np.float64(f32(a_val))
    rh = f32(r64)
    rl = f32(r64 - np.float64(rh))
    return float(rh), float(rl)


def _ap(t, off_elems, dims):
    """AP over tile t's buffer with free dims [[stride, n], ...] (elems)."""
    return bass.AP(tensor=t.tensor, offset=t.offset + off_elems, ap=[t.ap[0]] + dims)


def _build(rows, a_val):
    nc = bacc.Bacc("TRN2", target_bir_lowering=False)
    x_d = nc.dram_tensor("x", [rows, 16], dt.float32, kind="ExternalInput")
    # candidate-axis constants replicated R times: [32, R] row-major
    ck_d = nc.dram_tensor("ck", [4 * 32 * R], dt.float32, kind="ExternalInput")
    cf_d = nc.dram_tensor("cf", [48], dt.float32, kind="ExternalInput")
    ci_d = nc.dram_tensor("ci", [16], dt.int32, kind="ExternalInput")
    y_d = nc.dram_tensor("y", [rows, 16], dt.float32, kind="ExternalOutput")

    rh, rl = _div_consts(a_val)
    n_iters = rows // (128 * R)
    assert n_iters * 128 * R == rows
    rpp = rows // 128  # rows per partition (contiguous DMA mapping)
    assert rpp == R * n_iters

    # ck layout: sgn32R[0:32R] pcqR[32R:64R] ckqR[64R:96R] cccR[96R:128R]
    OS, OP, OK, OC = 0, 32 * R, 64 * R, 96 * R
    with tile.TileContext(nc) as tc:
        with contextlib.ExitStack() as ctx:
            singles = ctx.enter_context(tc.tile_pool(name="singles", bufs=1))
            ck_t = singles.tile([128, 4 * 32 * R], dt.float32)
            nc.sync.dma_start(out=ck_t, in_=bass.AP(tensor=ck_d, offset=0, ap=[[0, 128], [1, 4 * 32 * R]]))
            cf_t = singles.tile([128, 48], dt.float32)
            nc.sync.dma_start(out=cf_t, in_=bass.AP(tensor=cf_d, offset=0, ap=[[0, 128], [1, 48]]))
            ci_t = singles.tile([128, 16], dt.int32)
            nc.sync.dma_start(out=ci_t, in_=bass.AP(tensor=ci_d, offset=0, ap=[[0, 128], [1, 16]]))

            def ckb(off):  # [*, 32, R] broadcast of a candidate-axis const
                return _ap(ck_t, off, [[1, 32 * R]])

            work = ctx.enter_context(tc.tile_pool(name="work", bufs=2))

            for it in range(n_iters):
                # contiguous row mapping: partition P holds rows P*rpp + it*R .. +R
                row_off = it * R * 16  # within each partition's block, in elems
                x_t = work.tile([128, R, 16], dt.float32)
                nc.sync.dma_start(
                    out=x_t,
                    in_=bass.AP(tensor=x_d, offset=row_off, ap=[[rpp * 16, 128], [1, R * 16]]),
                )
                # ---------------- x/a via double-float multiply (validated exact)
                pD = work.tile([128, R, 16], dt.float32)
                nc.scalar.activation(out=pD, in_=x_t, func=Act.Copy, bias=0.0, scale=rh)
                xr = work.tile([128, R, 16], dt.float32)
                nc.scalar.activation(out=xr, in_=x_t, func=Act.Copy, bias=0.0, scale=rl)
                xs = work.tile([128, R, 16], dt.float32)
                nc.gpsimd.tensor_tensor(out=xs, in0=xr, in1=pD, op=Alu.add)
                if KSTAGE == 1:
                    nc.sync.dma_start(
                        out=bass.AP(tensor=y_d, offset=row_off, ap=[[rpp * 16, 128], [1, R * 16]]),
                        in_=xs)
                    continue

                # ---------------- frontend: fE, eE, u, dEO (natural [*, R, 16])
                h_t = xr  # in-place reuse
                nc.scalar.activation(out=h_t, in_=xs, func=Act.Copy, bias=MAGIC1, scale=0.5)
                fE = work.tile([128, R, 16], dt.float32)
                nc.scalar.activation(out=fE, in_=h_t, func=Act.Copy, bias=-MAGIC1, scale=1.0)
                eE = work.tile([128, R, 16], dt.float32)
                nc.vector.scalar_tensor_tensor(
                    out=eE, in0=fE, scalar=2.0, in1=xs, op0=Alu.mult, op1=Alu.subtract)
                dEO = work.tile([128, R, 16], dt.float32)
                nc.scalar.activation(out=dEO, in_=eE, func=Act.Sign, bias=0.0, scale=-1.0)
                SfE = work.tile([128, R], dt.float32)
                nc.vector.tensor_reduce(out=SfE, in_=fE, axis=AX.X, op=Alu.add)

                # transposed stacks [*, stack2, lane16, R]: WmT = (u, -u), W0T = (u-1/2, dEO)
                WmT = work.tile([128, 2, 16, R], dt.float32)
                u_tr = _ap(WmT, 0, [[R, 16], [1, R]])
                nc.scalar.activation(  # scatter-read transpose + Abs
                    out=u_tr, in_=_ap(eE, 0, [[1, 16], [16, R]]), func=Act.Abs, bias=0.0, scale=1.0)
                nc.scalar.activation(
                    out=_ap(WmT, 16 * R, [[R, 16], [1, R]]), in_=u_tr, func=Act.Copy, bias=0.0, scale=-1.0)
                W0T = work.tile([128, 2, 16, R], dt.float32)
                nc.scalar.activation(
                    out=_ap(W0T, 0, [[R, 16], [1, R]]), in_=u_tr, func=Act.Copy, bias=-0.5, scale=1.0)
                nc.scalar.activation(  # scatter-read transpose of dEO
                    out=_ap(W0T, 16 * R, [[R, 16], [1, R]]),
                    in_=_ap(dEO, 0, [[1, 16], [16, R]]), func=Act.Copy, bias=0.0, scale=1.0)

                # ---------------- WHT butterflies, [*, 2, 16, R], lane = rest*2M + c*M + Mi
                Wsrc = W0T
                wppA = work.tile([128, 2, 16, R], dt.float32, name="wppA")
                Wpp = [wppA, W0T]
                for t in range(4):
                    Wdst = Wpp[t % 2]
                    rest = 2 ** (3 - t)
                    M = 2 ** t
                    # stack (stride 16R) merges with rest (stride 2MR): 16 = rest*2M
                    i_lo = _ap(Wsrc, 0, [[2 * M * R, 2 * rest], [R, M], [1, R]])
                    i_hi = _ap(Wsrc, M * R, [[2 * M * R, 2 * rest], [R, M], [1, R]])
                    o_add = _ap(Wdst, 0, [[2 * M * R, 2 * rest], [2 * R, M], [1, R]])
                    o_sub = _ap(Wdst, R, [[2 * M * R, 2 * rest], [2 * R, M], [1, R]])
                    nc.gpsimd.tensor_tensor(out=o_add, in0=i_lo, in1=i_hi, op=Alu.add)
                    nc.gpsimd.tensor_tensor(out=o_sub, in0=i_lo, in1=i_hi, op=Alu.subtract)
                    Wsrc = Wdst
                W4 = Wsrc  # stack0 = Q[mt], stack1 = WdEO[mt] (lane stride R)
                if KSTAGE == 2:
                    nc.sync.dma_start(
                        out=bass.AP(tensor=y_d, offset=row_off, ap=[[rpp * 16, 128], [16, R], [1, 16]]),
                        in_=_ap(W4, 0, [[1, R], [R, 16]]))
                    continue

                # ---------------- max butterfly tables [*, 2, 32lane, R]
                # lane = rest*4M + c*2M + Mi*2 + s
                T1 = work.tile([128, 2, 32, R], dt.float32)
                # stage1 (M=1): max into (m1=0,s=0); copy (m1=1,s=c1); memset (m1=0,s=1)
                nc.vector.tensor_tensor(
                    out=_ap(T1, 0, [[4 * R, 16], [1, R]]),      # stack merged: 32R = 8*4R
                    in0=_ap(WmT, 0, [[2 * R, 16], [1, R]]),
                    in1=_ap(WmT, R, [[2 * R, 16], [1, R]]),
                    op=Alu.max)
                nc.scalar.activation(
                    out=_ap(T1, 2 * R, [[32 * R, 2], [4 * R, 8], [R, 2], [1, R]]),
                    in_=_ap(WmT, 0, [[16 * R, 2], [2 * R, 8], [R, 2], [1, R]]),
                    func=Act.Copy, bias=0.0, scale=1.0)
                nc.gpsimd.memset(_ap(T1, R, [[4 * R, 16], [1, R]]), BIGNEG)
                Tsrc = T1
                tppA = work.tile([128, 2, 32, R], dt.float32, name="tppA")
                Tpp = [tppA, T1, tppA]
                for t in range(1, 4):
                    Tdst = Tpp[t - 1]
                    rest = 2 ** (3 - t)
                    M = 2 ** t
                    # stack (stride 32R) merges with rest (stride 4MR): 32 = rest*4M
                    nc.vector.tensor_tensor(  # m_new=0, both s: contiguous (Mi,s) runs
                        out=_ap(Tdst, 0, [[4 * M * R, 2 * rest], [4 * R, M], [1, 2 * R]]),
                        in0=_ap(Tsrc, 0, [[4 * M * R, 2 * rest], [2 * R, M], [1, 2 * R]]),
                        in1=_ap(Tsrc, 2 * M * R, [[4 * M * R, 2 * rest], [2 * R, M], [1, 2 * R]]),
                        op=Alu.max)
                    nc.vector.tensor_tensor(  # m_new=1, s=0: max(c0 s0, c1 s1)
                        out=_ap(Tdst, 2 * R, [[4 * M * R, 2 * rest], [4 * R, M], [1, R]]),
                        in0=_ap(Tsrc, 0, [[4 * M * R, 2 * rest], [2 * R, M], [1, R]]),
                        in1=_ap(Tsrc, 2 * M * R + R, [[4 * M * R, 2 * rest], [2 * R, M], [1, R]]),
                        op=Alu.max)
                    nc.vector.tensor_tensor(  # m_new=1, s=1: max(c0 s1, c1 s0)
                        out=_ap(Tdst, 3 * R, [[4 * M * R, 2 * rest], [4 * R, M], [1, R]]),
                        in0=_ap(Tsrc, R, [[4 * M * R, 2 * rest], [2 * R, M], [1, R]]),
                        in1=_ap(Tsrc, 2 * M * R, [[4 * M * R, 2 * rest], [2 * R, M], [1, R]]),
                        op=Alu.max)
                    Tsrc = Tdst
                T4 = Tsrc  # [*, stack2, lane=mt*2+s, R]
                if KSTAGE == 3:
                    nc.sync.dma_start(
                        out=bass.AP(tensor=y_d, offset=row_off, ap=[[rpp * 16, 128], [16, R], [1, 16]]),
                        in_=_ap(T4, 0, [[1, R], [2 * R, 16]]))
                    continue

                # ---------------- assembly in [*, cand32, R] (cand = s*16 + mt)
                mx1 = work.tile([128, 32, R], dt.float32)
                nc.scalar.activation(  # cands s=0 read VmaxN slot s=1
                    out=_ap(mx1, 0, [[R, 16], [1, R]]),
                    in_=_ap(T4, 32 * R + R, [[2 * R, 16], [1, R]]),
                    func=Act.Copy, bias=1.0, scale=1.0)
                nc.scalar.activation(  # cands s=1 read VmaxN slot s=0
                    out=_ap(mx1, 16 * R, [[R, 16], [1, R]]),
                    in_=_ap(T4, 32 * R, [[2 * R, 16], [1, R]]),
                    func=Act.Copy, bias=1.0, scale=1.0)
                mx2 = work.tile([128, 32, R], dt.float32)
                nc.vector.tensor_tensor(
                    out=mx2,
                    in0=_ap(T4, 0, [[R, 2], [2 * R, 16], [1, R]]),  # VmaxU[s, mt]
                    in1=mx1, op=Alu.max)
                pe1 = mx1  # in-place: 4 - 4*M
                nc.scalar.activation(out=pe1, in_=mx2, func=Act.Copy, bias=4.0, scale=-4.0)

                # parity: odd = (SfE - WdEO[0]/4 + (1-2s)/4*WdEO[mt] + CKq) mod 2
                p1 = work.tile([128, 32, R], dt.float32)
                nc.scalar.activation(  # s=0 half: +WdEO/4
                    out=_ap(p1, 0, [[1, 16 * R]]),
                    in_=_ap(W4, 16 * R, [[1, 16 * R]]), func=Act.Copy, bias=0.0, scale=0.25)
                nc.scalar.activation(  # s=1 half: -WdEO/4
                    out=_ap(p1, 16 * R, [[1, 16 * R]]),
                    in_=_ap(W4, 16 * R, [[1, 16 * R]]), func=Act.Copy, bias=0.0, scale=-0.25)
                w4l = work.tile([128, R], dt.float32)
                nc.scalar.activation(  # -WdEO[0]/4
                    out=w4l, in_=_ap(W4, 16 * R, [[1, R]]), func=Act.Copy, bias=0.0, scale=-0.25)
                prow = work.tile([128, R], dt.float32)
                nc.gpsimd.tensor_tensor(out=prow, in0=w4l, in1=SfE, op=Alu.add)
                p2 = p1  # in-place
                nc.gpsimd.tensor_tensor(out=p2, in0=p1, in1=_ap(prow, 0, [[0, 32], [1, R]]), op=Alu.add)
                p4 = work.tile([128, 32, R], dt.float32)
                nc.gpsimd.tensor_tensor(out=p4, in0=p2, in1=ckb(OK), op=Alu.add)
                th = work.tile([128, 32, R], dt.float32)
                nc.scalar.activation(out=th, in_=p4, func=Act.Copy, bias=MAGIC1, scale=0.5)
                rr = work.tile([128, 32, R], dt.float32)
                nc.scalar.activation(out=rr, in_=th, func=Act.Copy, bias=-MAGIC1, scale=1.0)
                dd2 = th  # in-place
                nc.vector.scalar_tensor_tensor(
                    out=dd2, in0=p4, scalar=0.5, in1=rr, op0=Alu.mult, op1=Alu.subtract)
                odd = rr  # in-place
                nc.scalar.activation(out=odd, in_=dd2, func=Act.Square, bias=0.0, scale=2.0)

                pe2 = mx2  # in-place
                nc.vector.tensor_tensor(out=pe2, in0=pe1, in1=odd, op=Alu.mult)
                q1 = work.tile([128, 32, R], dt.float32, name="q1t")
                nc.scalar.activation(  # s=0 half: +Q
                    out=_ap(q1, 0, [[1, 16 * R]]),
                    in_=_ap(W4, 0, [[1, 16 * R]]), func=Act.Copy, bias=0.0, scale=1.0)
                nc.scalar.activation(  # s=1 half: -Q
                    out=_ap(q1, 16 * R, [[1, 16 * R]]),
                    in_=_ap(W4, 0, [[1, 16 * R]]), func=Act.Copy, bias=0.0, scale=-1.0)
                Dq = work.tile([128, 32, R], dt.float32)
                nc.gpsimd.tensor_tensor(out=Dq, in0=q1, in1=pe2, op=Alu.add)
                if KSTAGE == 4:
                    nc.sync.dma_start(
                        out=bass.AP(tensor=y_d, offset=row_off, ap=[[rpp * 16, 128], [16, R], [1, 16]]),
                        in_=_ap(Dq, 0, [[1, R], [R, 16]]))
                    continue

                # ---------------- argmin over candidates (strided-inner reduce)
                Dmin = work.tile([128, R], dt.float32)
                nc.vector.tensor_reduce(
                    out=Dmin, in_=_ap(Dq, 0, [[1, R], [R, 32]]), axis=AX.X, op=Alu.min)
                eq = q1  # in-place
                nc.vector.tensor_tensor(
                    out=eq, in0=Dq, in1=_ap(Dmin, 0, [[0, 32], [1, R]]), op=Alu.is_equal)
                m1k = pe2  # in-place
                nc.gpsimd.tensor_tensor(out=m1k, in0=eq, in1=ckb(OC), op=Alu.mult)
                km = work.tile([128, R], dt.float32)
                nc.vector.tensor_reduce(
                    out=km, in_=_ap(m1k, 0, [[1, R], [R, 32]]), axis=AX.X, op=Alu.min)
                oh2 = m1k  # in-place
                nc.gpsimd.tensor_tensor(out=oh2, in0=eq, in1=odd, op=Alu.mult)
                ods = work.tile([128, R], dt.float32)
                nc.vector.tensor_reduce(
                    out=ods, in_=_ap(oh2, 0, [[1, R], [R, 32]]), axis=AX.X, op=Alu.add)

                # ---------------- decode winner pattern b from CC bits
                ki = work.tile([128, R], dt.int32)
                nc.scalar.activation(out=ki, in_=km, func=Act.Copy, bias=2200000.0, scale=1.0)
                tsh = work.tile([128, R, 16], dt.int32)
                nc.vector.tensor_tensor(
                    out=tsh, in0=_ap(ki, 0, [[1, R], [0, 16]]),
                    in1=_ap(ci_t, 0, [[0, R], [1, 16]]), op=Alu.logical_shift_right)
                nc.vector.tensor_scalar(out=tsh, in0=tsh, scalar1=1, scalar2=None, op0=Alu.bitwise_and)
                b_t = work.tile([128, R, 16], dt.float32)
                nc.scalar.activation(out=b_t, in_=tsh, func=Act.Copy, bias=0.0, scale=1.0)
                if KSTAGE == 5:
                    nc.sync.dma_start(
                        out=bass.AP(tensor=y_d, offset=row_off, ap=[[rpp * 16, 128], [1, R * 16]]),
                        in_=b_t)
                    continue

                # ---------------- X, flip repair, output (natural [*, R, 16])
                x1 = work.tile([128, R, 16], dt.float32)
                nc.gpsimd.tensor_tensor(out=x1, in0=b_t, in1=dEO, op=Alu.mult)
                X_t = work.tile([128, R, 16], dt.float32)
                nc.vector.scalar_tensor_tensor(
                    out=X_t, in0=fE, scalar=2.0, in1=x1, op0=Alu.mult, op1=Alu.add)
                esel = x1  # in-place (x1 dead)
                nc.gpsimd.tensor_tensor(out=esel, in0=eE, in1=x1, op=Alu.add)
                ae = work.tile([128, R, 16], dt.float32)
                nc.scalar.activation(out=ae, in_=esel, func=Act.Abs, bias=0.0, scale=1.0)
                M16 = work.tile([128, R], dt.float32)
                nc.vector.tensor_reduce(out=M16, in_=ae, axis=AX.X, op=Alu.max)
                meq = b_t  # in-place (b dead)
                nc.vector.tensor_tensor(
                    out=meq, in0=ae, in1=_ap(M16, 0, [[1, R], [0, 16]]), op=Alu.is_equal)
                m2 = ae  # in-place
                nc.gpsimd.tensor_tensor(
                    out=m2, in0=meq, in1=_ap(cf_t, 0, [[0, R], [1, 16]]), op=Alu.mult)
                jm = work.tile([128, R], dt.float32)
                nc.vector.tensor_reduce(out=jm, in_=m2, axis=AX.X, op=Alu.min)
                mask1 = meq  # in-place
                nc.vector.tensor_tensor(
                    out=mask1, in0=_ap(cf_t, 0, [[0, R], [1, 16]]),
                    in1=_ap(jm, 0, [[1, R], [0, 16]]), op=Alu.is_equal)
                sgn = work.tile([128, R, 16], dt.float32)
                nc.scalar.activation(out=sgn, in_=esel, func=Act.Sign, bias=0.0, scale=1.0)
                u1 = sgn  # in-place
                nc.gpsimd.tensor_tensor(out=u1, in0=mask1, in1=sgn, op=Alu.mult)
                odm = work.tile([128, R], dt.float32)
                nc.scalar.activation(out=odm, in_=ods, func=Act.Copy, bias=0.0, scale=-2.0)
                u2 = mask1  # in-place
                nc.gpsimd.tensor_tensor(
                    out=u2, in0=u1, in1=_ap(odm, 0, [[1, R], [0, 16]]), op=Alu.mult)
                Xf = X_t  # in-place
                nc.gpsimd.tensor_tensor(out=Xf, in0=X_t, in1=u2, op=Alu.add)
                y_t = fE  # in-place reuse (fE dead)
                nc.scalar.activation(out=y_t, in_=Xf, func=Act.Copy, bias=0.0, scale=float(f32(a_val)))
                nc.sync.dma_start(
                    out=bass.AP(tensor=y_d, offset=row_off, ap=[[rpp * 16, 128], [1, R * 16]]),
                    in_=y_t,
                )
    nc.finalize()
    return nc


_CACHE = {}


def _get_nc(rows, a_val):
    key = (rows, a_val)
    if key not in _CACHE:
        _CACHE[key] = _build(rows, a_val)
    return _CACHE[key]


def _const_maps():
    ck = np.concatenate([
        np.repeat(SGN32, R), np.repeat(PCQ32, R), np.repeat(CKQ32, R), np.repeat(CCC, R),
    ]).astype(np.float32)
    cf = np.concatenate([I16C, CCC]).astype(np.float32)
    return ck, cf, PSHIFT


def kernel(x_in, C_rep, a):
    from concourse.bass_utils import run_bass_kernel_spmd

    x = np.asarray(x_in, dtype=np.float32)
    a_val = float(np.asarray(a).reshape(-1)[0])
    B = x.shape[0]
    rows = B // N_CORES
    assert rows * N_CORES == B

    xP = np.ascontiguousarray(x[:, JINV])
    nc = _get_nc(rows, a_val)
    ck, cf, ci = _const_maps()
    shards = xP.reshape(N_CORES, rows, 16)
    in_maps = [{"x": shards[i], "ck": ck, "cf": cf, "ci": ci} for i in range(N_CORES)]
    res = run_bass_kernel_spmd(nc, in_maps, core_ids=list(range(N_CORES)))
    yP = np.concatenate([res.results[i]["y"] for i in range(N_CORES)], axis=0)
    y = np.empty_like(yP)
    y[:, JINV] = yP
    return y.astype(np.float32)


if __name__ == "__main__":
    rng = np.random.default_rng(0)
    x = rng.standard_normal((262144, 16), dtype=np.float32)
    C = rng.integers(0, 5, size=(32, 16)).astype(np.float32)
    a = np.array([0.59460354], dtype=np.float32)
    y = kernel(x, C, a)
    print("ok", y.shape, y.dtype)


# revision 39
# speedup vs baseline: 1.1369x; 1.1369x over previous
"""Barnes-Wall (BW16) lattice quantizer for Trainium2, 8-core data-parallel.

Fast-Hadamard reformulation (validated bit-exact vs the jax reference in numpy):
  Per coordinate there are only two rounded candidates: the nearest even-lattice
  point E = 2*RNE(x/2) (error eE = E - x) and the nearest odd point O = E + dEO.
  Codeword k selects E or O per coordinate via its parity pattern b_k, and the 32
  patterns form the RM(1,4) code: with columns permuted so the code labels are
  position bits, b_k[p] = s XOR <m, p>.  Dropping row-common terms, the squared
  distance is sgn_s * WHT_m(|eE|-1/2), the rounded vector's parity comes from
  WHT_m(dEO), and the parity-repair penalty max|e| from max/min half-space
  tables built with a max-butterfly.  No [*,32,16] tensors anywhere.

Layout: butterfly/assembly tensors are stored transposed as [lane, R] with the
R row-blocks contiguous innermost, so every engine instruction streams runs of
R elements regardless of lane-shuffle strides.
"""
import os
import sys

sys.path.insert(0, "/opt/trn_rl_repo")
import contextlib

KSTAGE = int(os.environ.get("KSTAGE", "99"))  # debug bisection stage

import numpy as np

import concourse.bass as bass
import concourse.bacc as bacc
import concourse.mybir as mybir
import concourse.tile as tile

f32 = np.float32
MAGIC1 = float(f32(1.5 * 2.0**23))  # round-to-nearest-integer magic

dt = mybir.dt
Alu = mybir.AluOpType
Act = mybir.ActivationFunctionType
AX = mybir.AxisListType

N_CORES = 8
R = 32  # row blocks per iteration (rows/iter = 128*R)
BIGNEG = -1.0e30

# ---- host constants (derived from the fixed BW16 codebook; see module docstring)
_G = np.array([
    [1,1,1,1,0,1,0,1,1,0,0,1,0,0,0,0],
    [0,1,1,1,1,0,1,0,1,1,0,0,1,0,0,0],
    [0,0,1,1,1,1,0,1,0,1,1,0,0,1,0,0],
    [0,0,0,1,1,1,1,0,1,0,1,1,0,0,1,0],
    [1,1,1,1,1,1,1,1,1,1,1,1,1,1,1,1]], dtype=np.int64)


def _host_consts():
    import itertools
    G = _G
    bits_all = np.array(list(itertools.product([0, 1], repeat=5)), dtype=np.int64)
    Ci = bits_all @ G
    Bp = Ci % 2
    v = (G[0] + G[1] * 2 + G[2] * 4 + G[3] * 8)
    jinv = np.zeros(16, dtype=np.int64)
    for j in range(16):
        jinv[v[j]] = j
    bitrev = np.array([int(f"{m:04b}"[::-1], 2) for m in range(16)])
    orig_r = np.zeros(32, dtype=np.int64)
    for k in range(32):
        s, mt = divmod(k, 16)
        m = bitrev[mt]
        orig_r[k] = 16 * (m & 1) + 8 * ((m >> 1) & 1) + 4 * ((m >> 2) & 1) + 2 * ((m >> 3) & 1) + s
    Tk = np.array([int(np.sum(np.where(Bp[r] == 1, (Ci[r] - 1) // 2, Ci[r] // 2))) for r in range(32)])
    sgn32 = (1.0 - 2.0 * (np.arange(32) // 16)).astype(np.float32)
    pcq32 = (sgn32 * 0.25).astype(np.float32)
    ckq32 = np.array([
        Tk[orig_r[k]] + 4 - 4 * (1 - 2 * (k // 16)) * (1 if k % 16 == 0 else 0) + 256
        for k in range(32)], dtype=np.float32)
    # CC packs (original rank, permuted-codeword bits) so one masked-min both
    # breaks ties by original rank and yields the winner's bit pattern.
    bw = np.zeros(32, dtype=np.int64)
    for k in range(32):
        s, mt = divmod(k, 16)
        m = bitrev[mt]
        for p in range(16):
            if (s + bin(m & p).count("1")) % 2 == 1:
                bw[k] += 1 << p
    ccc = (orig_r * 65536 + bw - 2200000).astype(np.float32)
    i16c = (jinv - 32).astype(np.float32)
    pshift = np.arange(16, dtype=np.int32)
    return jinv, sgn32, pcq32, ckq32, ccc, i16c, pshift


JINV, SGN32, PCQ32, CKQ32, CCC, I16C, PSHIFT = _host_consts()


def _div_consts(a_val):
    r64 = 1.0 / np.float64(f32(a_val))
    rh = f32(r64)
    rl = f32(r64 - np.float64(rh))
    return float(rh), float(rl)


def _ap(t, off_elems, dims):
    """AP over tile t's buffer with free dims [[stride, n], ...] (elems)."""
    return bass.AP(tensor=t.tensor, offset=t.offset + off_elems, ap=[t.ap[0]] + dims)


def _build(rows, a_val):
    nc = bacc.Bacc("TRN2", target_bir_lowering=False)
    x_d = nc.dram_tensor("x", [rows, 16], dt.float32, kind="ExternalInput")
    # candidate-axis constants replicated 8x: [32, 8] row-major
    ck_d = nc.dram_tensor("ck", [2 * 32 * 8], dt.float32, kind="ExternalInput")
    cf_d = nc.dram_tensor("cf", [48], dt.float32, kind="ExternalInput")
    ci_d = nc.dram_tensor("ci", [16], dt.int32, kind="ExternalInput")
    y_d = nc.dram_tensor("y", [rows, 16], dt.float32, kind="ExternalOutput")

    rh, rl = _div_consts(a_val)
    n_iters = rows // (128 * R)
    assert n_iters * 128 * R == rows
    rpp = rows // 128  # rows per partition (contiguous DMA mapping)
    assert rpp == R * n_iters

    # ck layout: ckq8[0:256] ccc8[256:512]
    OK, OC = 0, 256
    with tile.TileContext(nc) as tc:
        with contextlib.ExitStack() as ctx:
            singles = ctx.enter_context(tc.tile_pool(name="singles", bufs=1))
            ck_t = singles.tile([128, 2 * 32 * 8], dt.float32)
            nc.sync.dma_start(out=ck_t, in_=bass.AP(tensor=ck_d, offset=0, ap=[[0, 128], [1, 2 * 32 * 8]]))
            cf_t = singles.tile([128, 48], dt.float32)
            nc.sync.dma_start(out=cf_t, in_=bass.AP(tensor=cf_d, offset=0, ap=[[0, 128], [1, 48]]))
            ci_t = singles.tile([128, 16], dt.int32)
            nc.sync.dma_start(out=ci_t, in_=bass.AP(tensor=ci_d, offset=0, ap=[[0, 128], [1, 16]]))

            def ckb(off):  # [*, 32, rblk4, 8] broadcast of a candidate-axis const
                return _ap(ck_t, off, [[8, 32], [0, R // 8], [1, 8]])

            work = ctx.enter_context(tc.tile_pool(name="work", bufs=3))

            for it in range(n_iters):
                # contiguous row mapping: partition P holds rows P*rpp + it*R .. +R
                row_off = it * R * 16  # within each partition's block, in elems
                x_t = work.tile([128, R, 16], dt.float32)
                nc.sync.dma_start(
                    out=x_t,
                    in_=bass.AP(tensor=x_d, offset=row_off, ap=[[rpp * 16, 128], [1, R * 16]]),
                )
                # ---------------- x/a via double-float multiply (validated exact)
                pD = work.tile([128, R, 16], dt.float32)
                nc.scalar.activation(out=pD, in_=x_t, func=Act.Copy, bias=0.0, scale=rh)
                xr = work.tile([128, R, 16], dt.float32)
                nc.scalar.activation(out=xr, in_=x_t, func=Act.Copy, bias=0.0, scale=rl)
                xs = work.tile([128, R, 16], dt.float32)
                nc.gpsimd.tensor_tensor(out=xs, in0=xr, in1=pD, op=Alu.add)
                if KSTAGE == 1:
                    nc.sync.dma_start(
                        out=bass.AP(tensor=y_d, offset=row_off, ap=[[rpp * 16, 128], [1, R * 16]]),
                        in_=xs)
                    continue

                # ---------------- frontend: fE, eE, u, dEO (natural [*, R, 16])
                h_t = xr  # in-place reuse
                nc.scalar.activation(out=h_t, in_=xs, func=Act.Copy, bias=MAGIC1, scale=0.5)
                fE = work.tile([128, R, 16], dt.float32)
                nc.scalar.activation(out=fE, in_=h_t, func=Act.Copy, bias=-MAGIC1, scale=1.0)
                eE = work.tile([128, R, 16], dt.float32)
                nc.vector.scalar_tensor_tensor(
                    out=eE, in0=fE, scalar=2.0, in1=xs, op0=Alu.mult, op1=Alu.subtract)
                dEO = pD  # in-place reuse (pD dead after xs)
                nc.scalar.activation(out=dEO, in_=eE, func=Act.Sign, bias=0.0, scale=-1.0)
                SfE = work.tile([128, R], dt.float32)
                nc.vector.tensor_reduce(out=SfE, in_=fE, axis=AX.X, op=Alu.add)

                # transposed stacks [*, stack2, lane16, R]: WmT = (u, -u), W0T = (u-1/2, dEO)
                WmT = work.tile([128, 2, 16, R], dt.float32)
                u_tr = _ap(WmT, 0, [[R, 16], [1, R]])
                nc.scalar.activation(  # scatter-read transpose + Abs
                    out=u_tr, in_=_ap(eE, 0, [[1, 16], [16, R]]), func=Act.Abs, bias=0.0, scale=1.0)
                nc.scalar.activation(
                    out=_ap(WmT, 16 * R, [[R, 16], [1, R]]), in_=u_tr, func=Act.Copy, bias=0.0, scale=-1.0)
                W0T = work.tile([128, 2, 16, R], dt.float32)
                nc.scalar.activation(
                    out=_ap(W0T, 0, [[R, 16], [1, R]]), in_=u_tr, func=Act.Copy, bias=-0.5, scale=1.0)
                nc.scalar.activation(  # scatter-read transpose of dEO
                    out=_ap(W0T, 16 * R, [[R, 16], [1, R]]),
                    in_=_ap(dEO, 0, [[1, 16], [16, R]]), func=Act.Copy, bias=0.0, scale=1.0)

                # ---------------- WHT butterflies, [*, 2, 16, R], lane = rest*2M + c*M + Mi
                Wsrc = W0T
                wppA = work.tile([128, 2, 16, R], dt.float32, name="wppA")
                Wpp = [wppA, W0T]
                for t in range(4):
                    Wdst = Wpp[t % 2]
                    rest = 2 ** (3 - t)
                    M = 2 ** t
                    # stack (stride 16R) merges with rest (stride 2MR): 16 = rest*2M
                    i_lo = _ap(Wsrc, 0, [[2 * M * R, 2 * rest], [R, M], [1, R]])
                    i_hi = _ap(Wsrc, M * R, [[2 * M * R, 2 * rest], [R, M], [1, R]])
                    o_add = _ap(Wdst, 0, [[2 * M * R, 2 * rest], [2 * R, M], [1, R]])
                    o_sub = _ap(Wdst, R, [[2 * M * R, 2 * rest], [2 * R, M], [1, R]])
                    eng = nc.vector if t % 2 == 0 else nc.gpsimd
                    eng2 = nc.gpsimd if t % 2 == 0 else nc.vector
                    eng.tensor_tensor(out=o_add, in0=i_lo, in1=i_hi, op=Alu.add)
                    eng2.tensor_tensor(out=o_sub, in0=i_lo, in1=i_hi, op=Alu.subtract)
                    Wsrc = Wdst
                W4 = Wsrc  # stack0 = Q[mt], stack1 = WdEO[mt] (lane stride R)
                if KSTAGE == 2:
                    nc.sync.dma_start(
                        out=bass.AP(tensor=y_d, offset=row_off, ap=[[rpp * 16, 128], [16, R], [1, 16]]),
                        in_=_ap(W4, 0, [[1, R], [R, 16]]))
                    continue

                # ---------------- max butterfly tables [*, 2, 32lane, R]
                # lane = rest*4M + c*2M + Mi*2 + s
                T1 = work.tile([128, 2, 32, R], dt.float32)
                # stage1 (M=1): max into (m1=0,s=0); copy (m1=1,s=c1); memset (m1=0,s=1)
                nc.vector.tensor_tensor(
                    out=_ap(T1, 0, [[4 * R, 16], [1, R]]),      # stack merged: 32R = 8*4R
                    in0=_ap(WmT, 0, [[2 * R, 16], [1, R]]),
                    in1=_ap(WmT, R, [[2 * R, 16], [1, R]]),
                    op=Alu.max)
                nc.scalar.activation(
                    out=_ap(T1, 2 * R, [[32 * R, 2], [4 * R, 8], [R, 2], [1, R]]),
                    in_=_ap(WmT, 0, [[16 * R, 2], [2 * R, 8], [R, 2], [1, R]]),
                    func=Act.Copy, bias=0.0, scale=1.0)
                nc.gpsimd.memset(_ap(T1, R, [[4 * R, 16], [1, R]]), BIGNEG)
                Tsrc = T1
                tppA = work.tile([128, 2, 32, R], dt.float32, name="tppA")
                Tpp = [tppA, T1, tppA]
                for t in range(1, 4):
                    Tdst = Tpp[t - 1]
                    rest = 2 ** (3 - t)
                    M = 2 ** t
                    # stack (stride 32R) merges with rest (stride 4MR): 32 = rest*4M
                    nc.vector.tensor_tensor(  # m_new=0, both s: contiguous (Mi,s) runs
                        out=_ap(Tdst, 0, [[4 * M * R, 2 * rest], [4 * R, M], [1, 2 * R]]),
                        in0=_ap(Tsrc, 0, [[4 * M * R, 2 * rest], [2 * R, M], [1, 2 * R]]),
                        in1=_ap(Tsrc, 2 * M * R, [[4 * M * R, 2 * rest], [2 * R, M], [1, 2 * R]]),
                        op=Alu.max)
                    nc.vector.tensor_tensor(  # m_new=1, s=0: max(c0 s0, c1 s1)
                        out=_ap(Tdst, 2 * R, [[4 * M * R, 2 * rest], [4 * R, M], [1, R]]),
                        in0=_ap(Tsrc, 0, [[4 * M * R, 2 * rest], [2 * R, M], [1, R]]),
                        in1=_ap(Tsrc, 2 * M * R + R, [[4 * M * R, 2 * rest], [2 * R, M], [1, R]]),
                        op=Alu.max)
                    nc.vector.tensor_tensor(  # m_new=1, s=1: max(c0 s1, c1 s0)
                        out=_ap(Tdst, 3 * R, [[4 * M * R, 2 * rest], [4 * R, M], [1, R]]),
                        in0=_ap(Tsrc, R, [[4 * M * R, 2 * rest], [2 * R, M], [1, R]]),
                        in1=_ap(Tsrc, 2 * M * R, [[4 * M * R, 2 * rest], [2 * R, M], [1, R]]),
                        op=Alu.max)
                    Tsrc = Tdst
                T4 = Tsrc  # [*, stack2, lane=mt*2+s, R]
                if KSTAGE == 3:
                    nc.sync.dma_start(
                        out=bass.AP(tensor=y_d, offset=row_off, ap=[[rpp * 16, 128], [16, R], [1, 16]]),
                        in_=_ap(T4, 0, [[1, R], [2 * R, 16]]))
                    continue

                # ---------------- assembly in [*, cand32, R] (cand = s*16 + mt)
                # parity: odd = (SfE - WdEO[0]/4 + (1-2s)/4*WdEO[mt] + CKq) mod 2
                p1 = work.tile([128, 32, R], dt.float32)
                nc.scalar.activation(  # s=0 half: +WdEO/4
                    out=_ap(p1, 0, [[1, 16 * R]]),
                    in_=_ap(W4, 16 * R, [[1, 16 * R]]), func=Act.Copy, bias=0.0, scale=0.25)
                nc.scalar.activation(  # s=1 half: -WdEO/4
                    out=_ap(p1, 16 * R, [[1, 16 * R]]),
                    in_=_ap(W4, 16 * R, [[1, 16 * R]]), func=Act.Copy, bias=0.0, scale=-0.25)
                w4l = work.tile([128, R], dt.float32)
                nc.scalar.activation(  # -WdEO[0]/4
                    out=w4l, in_=_ap(W4, 16 * R, [[1, R]]), func=Act.Copy, bias=0.0, scale=-0.25)
                prow = work.tile([128, R], dt.float32)
                nc.gpsimd.tensor_tensor(out=prow, in0=w4l, in1=SfE, op=Alu.add)
                p2 = p1  # in-place
                nc.gpsimd.tensor_tensor(out=p2, in0=p1, in1=_ap(prow, 0, [[0, 32], [1, R]]), op=Alu.add)
                p4 = work.tile([128, 32, R], dt.float32)
                nc.gpsimd.tensor_tensor(
                    out=_ap(p4, 0, [[R, 32], [8, R // 8], [1, 8]]),
                    in0=_ap(p1, 0, [[R, 32], [8, R // 8], [1, 8]]),
                    in1=ckb(OK), op=Alu.add)
                th = work.tile([128, 32, R], dt.float32)
                nc.scalar.activation(out=th, in_=p4, func=Act.Copy, bias=MAGIC1, scale=0.5)
                rr = work.tile([128, 32, R], dt.float32)
                nc.scalar.activation(out=rr, in_=th, func=Act.Copy, bias=-MAGIC1, scale=1.0)
                dd2 = th  # in-place
                nc.vector.scalar_tensor_tensor(
                    out=dd2, in0=p4, scalar=0.5, in1=rr, op0=Alu.mult, op1=Alu.subtract)
                odd = rr  # in-place
                nc.scalar.activation(out=odd, in_=dd2, func=Act.Square, bias=0.0, scale=2.0)

                mx1 = p1  # reuse buffer (p1/p2 dead after p4)
                nc.scalar.activation(  # cands s=0 read VmaxN slot s=1
                    out=_ap(mx1, 0, [[R, 16], [1, R]]),
                    in_=_ap(T4, 32 * R + R, [[2 * R, 16], [1, R]]),
                    func=Act.Copy, bias=1.0, scale=1.0)
                nc.scalar.activation(  # cands s=1 read VmaxN slot s=0
                    out=_ap(mx1, 16 * R, [[R, 16], [1, R]]),
                    in_=_ap(T4, 32 * R, [[2 * R, 16], [1, R]]),
                    func=Act.Copy, bias=1.0, scale=1.0)
                mx2 = p4  # reuse buffer (p4 dead after th/dd2)
                nc.vector.tensor_tensor(
                    out=mx2,
                    in0=_ap(T4, 0, [[R, 2], [2 * R, 16], [1, R]]),  # VmaxU[s, mt]
                    in1=mx1, op=Alu.max)
                pe1 = mx1  # in-place: 4 - 4*M
                nc.scalar.activation(out=pe1, in_=mx2, func=Act.Copy, bias=4.0, scale=-4.0)

                pe2 = mx2  # in-place
                nc.vector.tensor_tensor(out=pe2, in0=pe1, in1=odd, op=Alu.mult)
                q1 = work.tile([128, 32, R], dt.float32, name="q1t")
                nc.scalar.activation(  # s=0 half: +Q
                    out=_ap(q1, 0, [[1, 16 * R]]),
                    in_=_ap(W4, 0, [[1, 16 * R]]), func=Act.Copy, bias=0.0, scale=1.0)
                nc.scalar.activation(  # s=1 half: -Q
                    out=_ap(q1, 16 * R, [[1, 16 * R]]),
                    in_=_ap(W4, 0, [[1, 16 * R]]), func=Act.Copy, bias=0.0, scale=-1.0)
                Dq = th  # reuse buffer (th/dd2 dead after odd)
                nc.gpsimd.tensor_tensor(out=Dq, in0=q1, in1=pe2, op=Alu.add)
                if KSTAGE == 4:
                    nc.sync.dma_start(
                        out=bass.AP(tensor=y_d, offset=row_off, ap=[[rpp * 16, 128], [16, R], [1, 16]]),
                        in_=_ap(Dq, 0, [[1, R], [R, 16]]))
                    continue

                # ---------------- argmin over candidates (strided-inner reduce)
                Dmin = work.tile([128, R], dt.float32)
                nc.vector.tensor_reduce(
                    out=Dmin, in_=_ap(Dq, 0, [[1, R], [R, 32]]), axis=AX.X, op=Alu.min)
                eq = q1  # in-place
                nc.vector.tensor_tensor(
                    out=eq, in0=Dq, in1=_ap(Dmin, 0, [[0, 32], [1, R]]), op=Alu.is_equal)
                m1k = pe2  # in-place
                nc.gpsimd.tensor_tensor(
                    out=_ap(m1k, 0, [[R, 32], [8, R // 8], [1, 8]]),
                    in0=_ap(eq, 0, [[R, 32], [8, R // 8], [1, 8]]),
                    in1=ckb(OC), op=Alu.mult)
                km = work.tile([128, R], dt.float32)
                nc.vector.tensor_reduce(
                    out=km, in_=_ap(m1k, 0, [[1, R], [R, 32]]), axis=AX.X, op=Alu.min)
                oh2 = m1k  # in-place
                nc.gpsimd.tensor_tensor(out=oh2, in0=eq, in1=odd, op=Alu.mult)
                ods = work.tile([128, R], dt.float32)
                nc.vector.tensor_reduce(
                    out=ods, in_=_ap(oh2, 0, [[1, R], [R, 32]]), axis=AX.X, op=Alu.add)

                # ---------------- decode winner pattern b from CC bits
                ki = work.tile([128, R], dt.int32)
                nc.scalar.activation(out=ki, in_=km, func=Act.Copy, bias=2200000.0, scale=1.0)
                tsh = _ap(x_t, 0, [[16, R], [1, 16]]).bitcast(dt.int32)  # x_t dead
                nc.vector.tensor_tensor(
                    out=tsh, in0=_ap(ki, 0, [[1, R], [0, 16]]),
                    in1=_ap(ci_t, 0, [[0, R], [1, 16]]), op=Alu.logical_shift_right)
                nc.vector.tensor_scalar(out=tsh, in0=tsh, scalar1=1, scalar2=None, op0=Alu.bitwise_and)
                b_t = xs  # in-place reuse (xs dead after eE)
                nc.scalar.activation(out=b_t, in_=tsh, func=Act.Copy, bias=0.0, scale=1.0)
                if KSTAGE == 5:
                    nc.sync.dma_start(
                        out=bass.AP(tensor=y_d, offset=row_off, ap=[[rpp * 16, 128], [1, R * 16]]),
                        in_=b_t)
                    continue

                # ---------------- X, flip repair, output (natural [*, R, 16])
                x1 = work.tile([128, R, 16], dt.float32)
                nc.gpsimd.tensor_tensor(out=x1, in0=b_t, in1=dEO, op=Alu.mult)
                X_t = work.tile([128, R, 16], dt.float32)
                nc.vector.scalar_tensor_tensor(
                    out=X_t, in0=fE, scalar=2.0, in1=x1, op0=Alu.mult, op1=Alu.add)
                esel = x1  # in-place (x1 dead)
                nc.gpsimd.tensor_tensor(out=esel, in0=eE, in1=x1, op=Alu.add)
                ae = work.tile([128, R, 16], dt.float32, name="aeT")
                nc.scalar.activation(out=ae, in_=esel, func=Act.Abs, bias=0.0, scale=1.0)
                M16 = work.tile([128, R], dt.float32)
                nc.vector.tensor_reduce(out=M16, in_=ae, axis=AX.X, op=Alu.max)
                meq = b_t  # in-place (b dead)
                nc.vector.tensor_tensor(
                    out=meq, in0=ae, in1=_ap(M16, 0, [[1, R], [0, 16]]), op=Alu.is_equal)
                m2 = ae  # in-place
                nc.gpsimd.tensor_tensor(
                    out=m2, in0=meq, in1=_ap(cf_t, 0, [[0, R], [1, 16]]), op=Alu.mult)
                jm = work.tile([128, R], dt.float32)
                nc.vector.tensor_reduce(out=jm, in_=m2, axis=AX.X, op=Alu.min)
                mask1 = meq  # in-place
                nc.vector.tensor_tensor(
                    out=mask1, in0=_ap(cf_t, 0, [[0, R], [1, 16]]),
                    in1=_ap(jm, 0, [[1, R], [0, 16]]), op=Alu.is_equal)
                sgn = xr  # in-place reuse (h_t dead after fE)
                nc.scalar.activation(out=sgn, in_=esel, func=Act.Sign, bias=0.0, scale=1.0)
                u1 = sgn  # in-place
                nc.gpsimd.tensor_tensor(out=u1, in0=mask1, in1=sgn, op=Alu.mult)
                odm = work.tile([128, R], dt.float32)
                nc.scalar.activation(out=odm, in_=ods, func=Act.Copy, bias=0.0, scale=-2.0)
                u2 = mask1  # in-place
                nc.gpsimd.tensor_tensor(
                    out=u2, in0=u1, in1=_ap(odm, 0, [[1, R], [0, 16]]), op=Alu.mult)
                Xf = X_t  # in-place
                nc.gpsimd.tensor_tensor(out=Xf, in0=X_t, in1=u2, op=Alu.add)
                y_t = fE  # in-place reuse (fE dead)
                nc.scalar.activation(out=y_t, in_=Xf, func=Act.Copy, bias=0.0, scale=float(f32(a_val)))
                nc.sync.dma_start(
                    out=bass.AP(tensor=y_d, offset=row_off, ap=[[rpp * 16, 128], [1, R * 16]]),
                    in_=y_t,
                )
    nc.finalize()
    return nc


_CACHE = {}


def _get_nc(rows, a_val):
    key = (rows, a_val)
    if key not in _CACHE:
        _CACHE[key] = _build(rows, a_val)
    return _CACHE[key]


def _const_maps():
    ck = np.concatenate([np.repeat(CKQ32, 8), np.repeat(CCC, 8)]).astype(np.float32)
    cf = np.concatenate([I16C, CCC]).astype(np.float32)
    return ck, cf, PSHIFT


def kernel(x_in, C_rep, a):
    from concourse.bass_utils import run_bass_kernel_spmd

    x = np.asarray(x_in, dtype=np.float32)
    a_val = float(np.asarray(a).reshape(-1)[0])
    B = x.shape[0]
    rows = B // N_CORES
    assert rows * N_CORES == B

    xP = np.ascontiguousarray(x[:, JINV])
    nc = _get_nc(rows, a_val)
    ck, cf, ci = _const_maps()
    shards = xP.reshape(N_CORES, rows, 16)
    in_maps = [{"x": shards[i], "ck": ck, "cf": cf, "ci": ci} for i in range(N_CORES)]
    res = run_bass_kernel_spmd(nc, in_maps, core_ids=list(range(N_CORES)))
    yP = np.concatenate([res.results[i]["y"] for i in range(N_CORES)], axis=0)
    y = np.empty_like(yP)
    y[:, JINV] = yP
    return y.astype(np.float32)


if __name__ == "__main__":
    rng = np.random.default_rng(0)
    x = rng.standard_normal((262144, 16), dtype=np.float32)
    C = rng.integers(0, 5, size=(32, 16)).astype(np.float32)
    a = np.array([0.59460354], dtype=np.float32)
    y = kernel(x, C, a)
    print("ok", y.shape, y.dtype)
